# revision 44
# baseline (speedup 1.0000x reference)
"""Bahdanau attention on 8 TRN2 NeuronCores, data-parallel over batch.

Reference computation (B=32, S=2048, H=U=1024):
    proj_v = values @ W1 + b1                 # [B,S,U]
    proj_q = query @ W2 + b2                  # [B,1,U]
    score  = tanh(proj_v + proj_q) @ V + bv   # [B,S,1]
    attw   = softmax(score, axis=1)
    ctx    = sum(attw * values, axis=1)       # [B,H]

Kernel strategy (per core, 4 batches):
  - values pre-transposed on host to [b, H, S] bf16 so the contraction dim
    (h) lands on SBUF partitions; W1 stays resident as the stationary matmul
    operand.  proj is produced transposed [u, s], which makes proj_q a
    per-partition bias fused into the tanh activation.
  - score via PE matvec with V over u-chunks; exp on ScalarE with accum_out
    giving the softmax denominator for free.  Scores are O(1) so the softmax
    runs unnormalized in a single pass (no max subtraction, no second read
    of values); bv drops out entirely by shift invariance.
  - exp(score) row is broadcast to 128 partitions with a rank-1 matmul and
    fused into sum(e^s * values) via the affine_mul_reduce DVE op on the
    same [h, s] tiles the big matmul consumed.
  - per-batch epilogue (softmax normalize + output DMA) overlaps the next
    batch's compute; context leaves via a PE transpose so the final DMA is
    contiguous.
"""

import numpy as np
import ml_dtypes

B, S, H, U = 32, 2048, 1024, 1024
NCORES = 8
BPC = B // NCORES  # batches per core
SC = 512           # s-chunk (matmul moving free dim)
NSC = S // SC
NK = H // 128      # contraction chunks
NJ = U // 128      # u chunks

BF16 = ml_dtypes.bfloat16

_CACHE = {}
LAST_RESULTS = None
RUN_KWARGS = {}


def _build_nc():
    import concourse.bacc as bacc
    import concourse.mybir as mybir
    import concourse.tile as tile
    from concourse import masks

    dt = mybir.dt
    f32 = dt.float32
    bf16 = dt.bfloat16
    Alu = mybir.AluOpType
    Act = mybir.ActivationFunctionType

    nc = bacc.Bacc("TRN2", target_bir_lowering=False, debug=False,
                   num_devices=NCORES)

    vT = nc.dram_tensor("vT", [BPC, H, S], bf16, kind="ExternalInput")
    w1d = nc.dram_tensor("w1d", [NK, 128, U], bf16, kind="ExternalInput")
    w2d = nc.dram_tensor("w2d", [NK, 128, U], bf16, kind="ExternalInput")
    qTd = nc.dram_tensor("qTd", [NK, 128, BPC], bf16, kind="ExternalInput")
    b12d = nc.dram_tensor("b12d", [128, NJ], f32, kind="ExternalInput")
    vvd = nc.dram_tensor("vvd", [128, NJ], bf16, kind="ExternalInput")
    ctx_out = nc.dram_tensor("ctx_out", [BPC, H], f32, kind="ExternalOutput")
    attw_out = nc.dram_tensor("attw_out", [BPC, S], f32, kind="ExternalOutput")

    def w_slice(w_sb, k, j):
        return w_sb[:, k * U + j * 128:k * U + (j + 1) * 128]

    with tile.TileContext(nc) as tc:
        with (
            tc.tile_pool(name="const", bufs=1) as cpool,
            tc.tile_pool(name="vt", bufs=24) as vtpool,
            tc.tile_pool(name="tanh", bufs=14) as thpool,
            tc.tile_pool(name="scratch", bufs=2) as scpool,
            tc.tile_pool(name="pp", bufs=4, space="PSUM") as pp,
            tc.tile_pool(name="sp", bufs=2, space="PSUM") as sp,
            tc.tile_pool(name="wp", bufs=2, space="PSUM") as wp,
        ):
            # ---- resident tiles -----------------------------------------
            w1_sb = cpool.tile([128, NK * U], bf16)
            w2_sb = cpool.tile([128, NK * U], bf16)
            qT_sb = cpool.tile([128, NK * BPC], bf16)
            b12_sb = cpool.tile([128, NJ], f32)
            vv_sb = cpool.tile([128, NJ], bf16)
            ones_bf = cpool.tile([128, 128], bf16)
            ones_f32 = cpool.tile([1, 128], f32)
            ident = cpool.tile([128, 128], f32)
            bias_full = cpool.tile([128, NJ * BPC], f32)
            w_rows = []
            wg_rows = []
            wout_rows = []
            for b in range(BPC):
                wrow = cpool.tile([128, NSC * 128], bf16, tag=f"wrow{b}")
                w_rows.append(wrow)
                wgrow = cpool.tile([1, S], bf16, tag=f"wgrow{b}")
                wg_rows.append(wgrow)
                worow = cpool.tile([1, S], f32, tag=f"worow{b}")
                wout_rows.append(worow)
            lparts = cpool.tile([128, BPC * 8], f32)
            lsum4 = cpool.tile([128, BPC], f32)
            lrow4 = cpool.tile([1, 4 * BPC], f32)
            l_row = cpool.tile([1, BPC], f32)
            linv_row = cpool.tile([1, BPC], f32)
            linv_rep = cpool.tile([128, BPC], f32)
            ctx_all = cpool.tile([128, BPC * NK], f32)

            # ---- input DMAs: W1 + first value tiles first ---------------
            vts0 = []
            for k in range(NK):
                nc.sync.dma_start(w1_sb[:, k * U:(k + 1) * U], w1d[k])
                t = vtpool.tile([128, SC], bf16, tag="vt")
                nc.sync.dma_start(t[:], vT[0, k * 128:(k + 1) * 128, 0:SC])
                vts0.append(t)
            nc.gpsimd.memset(ones_bf[:], 1.0)
            nc.gpsimd.memset(lparts[:], 0.0)
            nc.gpsimd.memset(ones_f32[:], 1.0)
            masks.make_identity(nc, ident[:])
            for k in range(NK):
                nc.sync.dma_start(qT_sb[:, k * BPC:(k + 1) * BPC], qTd[k])
            for k in range(NK):
                nc.sync.dma_start(w2_sb[:, k * U:(k + 1) * U], w2d[k])
            nc.sync.dma_start(b12_sb[:], b12d[:])
            nc.sync.dma_start(vv_sb[:], vvd[:])

            # ---- bias = W2^T q + (b1 + b2), [u_part, j*BPC + b] ----------
            for j in range(NJ):
                bp = wp.tile([128, BPC], mybir.dt.float32, tag="wrep")
                for k in range(NK):
                    nc.tensor.matmul(
                        bp[:], w_slice(w2_sb, k, j),
                        qT_sb[:, k * BPC:(k + 1) * BPC],
                        start=(k == 0), stop=(k == NK - 1),
                    )
                nc.vector.tensor_scalar(
                    bias_full[:, j * BPC:(j + 1) * BPC], bp[:],
                    b12_sb[:, j:j + 1], None, Alu.add,
                )

            # ---- main loop ----------------------------------------------
            def chunks_for(b):
                if b < BPC - 1:
                    return [(i * SC, SC) for i in range(NSC)]
                return ([(i * SC, SC) for i in range(NSC - 1)]
                        + [(S - SC, SC // 2), (S - SC // 2, SC // 2)])

            for b in range(BPC):
                for ci, (s_off, s_len) in enumerate(chunks_for(b)):
                    Q = s_len // 4
                    if b == 0 and ci == 0:
                        vts = vts0
                    else:
                        vts = []
                        for k in range(NK):
                            t = vtpool.tile([128, s_len], bf16, tag="vt")
                            nc.sync.dma_start(
                                t[:], vT[b, k * 128:(k + 1) * 128,
                                         s_off:s_off + s_len])
                            vts.append(t)

                    sps = sp.tile([128, Q], mybir.dt.float32, tag="score")
                    tanhs = []
                    nhalf = (s_len + 511) // 512
                    for j in range(NJ):
                        pj = pp.tile([128, s_len], mybir.dt.float32,
                                     tag="proj")
                        for h in range(nhalf):
                            c0, c1 = h * 512, min((h + 1) * 512, s_len)
                            for k in range(NK):
                                nc.tensor.matmul(
                                    pj[:, c0:c1], w_slice(w1_sb, k, j),
                                    vts[k][:, c0:c1],
                                    start=(k == 0), stop=(k == NK - 1),
                                )
                        tj = thpool.tile([128, s_len], bf16, tag="tanh")
                        nc.scalar.activation(
                            tj[:], pj[:], Act.Tanh,
                            bias=bias_full[:, j * BPC + b:j * BPC + b + 1],
                        )
                        tanhs.append(tj)

                    for j in range(NJ):
                        for g in range(4):
                            nc.tensor.matmul(
                                sps[32 * g:32 * g + 1, :],
                                vv_sb[:, j:j + 1],
                                tanhs[j][:, g * Q:(g + 1) * Q],
                                start=(j == 0), stop=(j == NJ - 1),
                                tile_position=(0, 32 * g),
                                skip_group_check=True,
                            )

                    qoff = s_off // 4
                    for g in range(4):
                        nc.scalar.activation(
                            w_rows[b][32 * g:32 * g + 1, qoff:qoff + Q],
                            sps[32 * g:32 * g + 1, :],
                            Act.Exp,
                            accum_out=lparts[32 * g:32 * g + 1,
                                             b * 8 + ci:b * 8 + ci + 1],
                        )
                    for g in range(4):
                        nc.sync.dma_start(
                            wg_rows[b][:, s_off + g * Q:
                                       s_off + (g + 1) * Q],
                            w_rows[b][32 * g:32 * g + 1, qoff:qoff + Q])
                    wr = scpool.tile([128, s_len], bf16, tag="wrb")
                    nc.gpsimd.partition_broadcast(
                        wr[:], wg_rows[b][:, s_off:s_off + s_len])

                    def amr_block(ci=ci, vts=vts, wr=wr,
                                  s_len=s_len):
                        if ci == 0:
                            for k in range(NK):
                                junk = scpool.tile([128, s_len], bf16,
                                                   tag="junk")
                                nc.vector.affine_mul_reduce(
                                    junk[:], ctx_all[:, b * NK + k:
                                                     b * NK + k + 1],
                                    vts[k][:], wr[:], 1.0, 0.0)
                        else:
                            parts = scpool.tile([128, NK],
                                                mybir.dt.float32,
                                                tag="parts")
                            for k in range(NK):
                                junk = scpool.tile([128, s_len], bf16,
                                                   tag="junk")
                                nc.vector.affine_mul_reduce(
                                    junk[:], parts[:, k:k + 1],
                                    vts[k][:], wr[:], 1.0, 0.0)
                            acc = ctx_all[:, b * NK:(b + 1) * NK]
                            nc.vector.tensor_tensor(acc, acc, parts[:],
                                                    Alu.add)

                    if ci < len(chunks_for(b)) - 1:
                        amr_block()
                    else:
                        deferred_amr = amr_block

                # ---- per-batch epilogue, overlaps next batch ------------
                nc.vector.tensor_reduce(
                    lsum4[:, b:b + 1],
                    lparts[:, b * 8:(b + 1) * 8],
                    mybir.AxisListType.X, Alu.add)
                for g in range(4):
                    nc.sync.dma_start(
                        lrow4[:, 4 * b + g:4 * b + g + 1],
                        lsum4[32 * g:32 * g + 1, b:b + 1])
                nc.vector.tensor_reduce(
                    l_row[:, b:b + 1],
                    lrow4[:, 4 * b:4 * (b + 1)],
                    mybir.AxisListType.X, Alu.add)
                nc.vector.reciprocal(linv_row[:, b:b + 1],
                                     l_row[:, b:b + 1])

                lrb = wp.tile([128, 1], mybir.dt.float32, tag="wrep")
                nc.tensor.matmul(lrb[:], ones_f32[:],
                                 linv_row[:, b:b + 1],
                                 start=True, stop=True)
                nc.vector.tensor_copy(linv_rep[:, b:b + 1], lrb[:])
                nc.scalar.activation(
                    wout_rows[b][:], wg_rows[b][:], Act.Copy,
                    scale=linv_row[:, b:b + 1])
                nc.sync.dma_start(attw_out[b:b + 1, :], wout_rows[b][:])
                deferred_amr()
                ctxt = wp.tile([NK, 128], mybir.dt.float32, tag="wrep")
                nc.tensor.transpose(
                    ctxt[:], ctx_all[:, b * NK:(b + 1) * NK], ident[:])
                ctxs = scpool.tile([NK, 128], mybir.dt.float32,
                                   tag=f"ctxs{b}")
                nc.vector.tensor_scalar(
                    ctxs[:], ctxt[:], linv_rep[0:NK, b:b + 1], None,
                    Alu.mult)
                nc.sync.dma_start(
                    ctx_out[b:b + 1, :].rearrange("b (k p) -> k (b p)",
                                                  k=NK),
                    ctxs[:],
                )

    nc.compile()
    return nc


def _prep_inputs(query, values, W1, b1, W2, b2, V, bv):
    """Host-side shard prep. bv is dropped: softmax is shift-invariant."""
    w1_8 = np.ascontiguousarray(W1.astype(BF16).reshape(NK, 128, U))
    w2_8 = np.ascontiguousarray(W2.astype(BF16).reshape(NK, 128, U))
    b12 = np.ascontiguousarray((b1 + b2).astype(np.float32)
                               .reshape(NJ, 128).T)
    vv = np.ascontiguousarray(V[:, 0].astype(BF16).reshape(NJ, 128).T)
    in_maps = []
    for c in range(NCORES):
        bs = slice(c * BPC, (c + 1) * BPC)
        vTa = np.ascontiguousarray(
            np.swapaxes(values[bs].astype(BF16), 1, 2))
        qT = np.ascontiguousarray(
            query[bs].T.astype(BF16).reshape(NK, 128, BPC))
        in_maps.append({
            "vT": vTa, "w1d": w1_8, "w2d": w2_8, "qTd": qT,
            "b12d": b12, "vvd": vv,
        })
    return in_maps


def kernel(query, values, W1, b1, W2, b2, V, bv):
    global LAST_RESULTS
    from concourse.bass_utils import run_bass_kernel_spmd

    if "nc" not in _CACHE:
        _CACHE["nc"] = _build_nc()
    nc = _CACHE["nc"]

    query = np.asarray(query)
    values = np.asarray(values)
    in_maps = _prep_inputs(query, values, np.asarray(W1), np.asarray(b1),
                           np.asarray(W2), np.asarray(b2), np.asarray(V),
                           np.asarray(bv))

    res = run_bass_kernel_spmd(nc, in_maps, core_ids=list(range(NCORES)),
                               **RUN_KWARGS)
    LAST_RESULTS = res

    ctx = np.concatenate(
        [np.asarray(r["ctx_out"]) for r in res.results], axis=0)
    attw = np.concatenate(
        [np.asarray(r["attw_out"]) for r in res.results], axis=0)
    return (np.ascontiguousarray(ctx, dtype=np.float32),
            np.ascontiguousarray(attw, dtype=np.float32)[..., None])


# revision 45
# speedup vs baseline: 1.0129x; 1.0129x over previous
"""Bahdanau attention on 8 TRN2 NeuronCores, data-parallel over batch.

Reference computation (B=32, S=2048, H=U=1024):
    proj_v = values @ W1 + b1                 # [B,S,U]
    proj_q = query @ W2 + b2                  # [B,1,U]
    score  = tanh(proj_v + proj_q) @ V + bv   # [B,S,1]
    attw   = softmax(score, axis=1)
    ctx    = sum(attw * values, axis=1)       # [B,H]

Kernel strategy (per core, 4 batches):
  - values pre-transposed on host to [b, H, S] bf16 so the contraction dim
    (h) lands on SBUF partitions; W1 stays resident as the stationary matmul
    operand.  proj is produced transposed [u, s], which makes proj_q a
    per-partition bias fused into the tanh activation.
  - score via PE matvec with V over u-chunks; exp on ScalarE with accum_out
    giving the softmax denominator for free.  Scores are O(1) so the softmax
    runs unnormalized in a single pass (no max subtraction, no second read
    of values); bv drops out entirely by shift invariance.
  - exp(score) row is broadcast to 128 partitions with a rank-1 matmul and
    fused into sum(e^s * values) via the affine_mul_reduce DVE op on the
    same [h, s] tiles the big matmul consumed.
  - per-batch epilogue (softmax normalize + output DMA) overlaps the next
    batch's compute; context leaves via a PE transpose so the final DMA is
    contiguous.
"""

import numpy as np
import ml_dtypes

B, S, H, U = 32, 2048, 1024, 1024
NCORES = 8
BPC = B // NCORES  # batches per core
SC = 512           # s-chunk (matmul moving free dim)
NSC = S // SC
NK = H // 128      # contraction chunks
NJ = U // 128      # u chunks

BF16 = ml_dtypes.bfloat16

_CACHE = {}
LAST_RESULTS = None
RUN_KWARGS = {}


def _build_nc():
    import concourse.bacc as bacc
    import concourse.mybir as mybir
    import concourse.tile as tile
    from concourse import masks

    dt = mybir.dt
    f32 = dt.float32
    bf16 = dt.bfloat16
    Alu = mybir.AluOpType
    Act = mybir.ActivationFunctionType

    nc = bacc.Bacc("TRN2", target_bir_lowering=False, debug=False,
                   num_devices=NCORES)

    vT = nc.dram_tensor("vT", [BPC, H, S], bf16, kind="ExternalInput")
    w1d = nc.dram_tensor("w1d", [NK, 128, U], bf16, kind="ExternalInput")
    w2d = nc.dram_tensor("w2d", [NK, 128, U], bf16, kind="ExternalInput")
    qTd = nc.dram_tensor("qTd", [NK, 128, BPC], bf16, kind="ExternalInput")
    b12d = nc.dram_tensor("b12d", [128, NJ], f32, kind="ExternalInput")
    vvd = nc.dram_tensor("vvd", [128, NJ], bf16, kind="ExternalInput")
    ctx_out = nc.dram_tensor("ctx_out", [BPC, H], f32, kind="ExternalOutput")
    attw_out = nc.dram_tensor("attw_out", [BPC, S], f32, kind="ExternalOutput")

    def w_slice(w_sb, k, j):
        return w_sb[:, k * U + j * 128:k * U + (j + 1) * 128]

    with tile.TileContext(nc) as tc:
        with (
            tc.tile_pool(name="const", bufs=1) as cpool,
            tc.tile_pool(name="vt", bufs=24) as vtpool,
            tc.tile_pool(name="tanh", bufs=14) as thpool,
            tc.tile_pool(name="scratch", bufs=2) as scpool,
            tc.tile_pool(name="pp", bufs=5, space="PSUM") as pp,
            tc.tile_pool(name="sp", bufs=1, space="PSUM") as sp,
            tc.tile_pool(name="wp", bufs=2, space="PSUM") as wp,
        ):
            # ---- resident tiles -----------------------------------------
            w1_sb = cpool.tile([128, NK * U], bf16)
            w2_sb = cpool.tile([128, NK * U], bf16)
            qT_sb = cpool.tile([128, NK * BPC], bf16)
            b12_sb = cpool.tile([128, NJ], f32)
            vv_sb = cpool.tile([128, NJ], bf16)
            ones_bf = cpool.tile([128, 128], bf16)
            ones_f32 = cpool.tile([1, 128], f32)
            ident = cpool.tile([128, 128], f32)
            bias_full = cpool.tile([128, NJ * BPC], f32)
            w_rows = []
            wg_rows = []
            wout_rows = []
            for b in range(BPC):
                wrow = cpool.tile([128, NSC * 128], bf16, tag=f"wrow{b}")
                w_rows.append(wrow)
                wgrow = cpool.tile([1, S], bf16, tag=f"wgrow{b}")
                wg_rows.append(wgrow)
                worow = cpool.tile([1, S], f32, tag=f"worow{b}")
                wout_rows.append(worow)
            lparts = cpool.tile([128, BPC * 8], f32)
            lsum4 = cpool.tile([128, BPC], f32)
            lrow4 = cpool.tile([1, 4 * BPC], f32)
            l_row = cpool.tile([1, BPC], f32)
            linv_row = cpool.tile([1, BPC], f32)
            linv_rep = cpool.tile([128, BPC], f32)
            ctx_all = cpool.tile([128, BPC * NK], f32)

            # ---- input DMAs: W1 + first value tiles first ---------------
            vts0 = []
            for k in range(NK):
                nc.sync.dma_start(w1_sb[:, k * U:(k + 1) * U], w1d[k])
                t = vtpool.tile([128, SC], bf16, tag="vt")
                nc.sync.dma_start(t[:], vT[0, k * 128:(k + 1) * 128, 0:SC])
                vts0.append(t)
            nc.gpsimd.memset(ones_bf[:], 1.0)
            nc.gpsimd.memset(lparts[:], 0.0)
            nc.gpsimd.memset(ones_f32[:], 1.0)
            masks.make_identity(nc, ident[:])
            for k in range(NK):
                nc.sync.dma_start(qT_sb[:, k * BPC:(k + 1) * BPC], qTd[k])
            for k in range(NK):
                nc.sync.dma_start(w2_sb[:, k * U:(k + 1) * U], w2d[k])
            nc.sync.dma_start(b12_sb[:], b12d[:])
            nc.sync.dma_start(vv_sb[:], vvd[:])

            # ---- bias = W2^T q + (b1 + b2), [u_part, j*BPC + b] ----------
            for j in range(NJ):
                bp = wp.tile([128, BPC], mybir.dt.float32, tag="wrep")
                for k in range(NK):
                    nc.tensor.matmul(
                        bp[:], w_slice(w2_sb, k, j),
                        qT_sb[:, k * BPC:(k + 1) * BPC],
                        start=(k == 0), stop=(k == NK - 1),
                    )
                nc.vector.tensor_scalar(
                    bias_full[:, j * BPC:(j + 1) * BPC], bp[:],
                    b12_sb[:, j:j + 1], None, Alu.add,
                )

            # ---- main loop ----------------------------------------------
            def chunks_for(b):
                if b < BPC - 1:
                    return [(i * SC, SC) for i in range(NSC)]
                return ([(i * SC, SC) for i in range(NSC - 1)]
                        + [(S - SC, SC // 2), (S - SC // 2, SC // 2)])

            for b in range(BPC):
                for ci, (s_off, s_len) in enumerate(chunks_for(b)):
                    Q = s_len // 4
                    if b == 0 and ci == 0:
                        vts = vts0
                    else:
                        vts = []
                        for k in range(NK):
                            t = vtpool.tile([128, s_len], bf16, tag="vt")
                            nc.sync.dma_start(
                                t[:], vT[b, k * 128:(k + 1) * 128,
                                         s_off:s_off + s_len])
                            vts.append(t)

                    sps = sp.tile([128, Q], mybir.dt.float32, tag="score")
                    tanhs = []
                    nhalf = (s_len + 511) // 512
                    for j in range(NJ):
                        pj = pp.tile([128, s_len], mybir.dt.float32,
                                     tag="proj")
                        for h in range(nhalf):
                            c0, c1 = h * 512, min((h + 1) * 512, s_len)
                            for k in range(NK):
                                nc.tensor.matmul(
                                    pj[:, c0:c1], w_slice(w1_sb, k, j),
                                    vts[k][:, c0:c1],
                                    start=(k == 0), stop=(k == NK - 1),
                                )
                        tj = thpool.tile([128, s_len], bf16, tag="tanh")
                        nc.scalar.activation(
                            tj[:], pj[:], Act.Tanh,
                            bias=bias_full[:, j * BPC + b:j * BPC + b + 1],
                        )
                        tanhs.append(tj)

                    for j in range(NJ):
                        for g in range(4):
                            nc.tensor.matmul(
                                sps[32 * g:32 * g + 1, :],
                                vv_sb[:, j:j + 1],
                                tanhs[j][:, g * Q:(g + 1) * Q],
                                start=(j == 0), stop=(j == NJ - 1),
                                tile_position=(0, 32 * g),
                                skip_group_check=True,
                            )

                    qoff = s_off // 4
                    for g in range(4):
                        nc.scalar.activation(
                            w_rows[b][32 * g:32 * g + 1, qoff:qoff + Q],
                            sps[32 * g:32 * g + 1, :],
                            Act.Exp,
                            accum_out=lparts[32 * g:32 * g + 1,
                                             b * 8 + ci:b * 8 + ci + 1],
                        )
                    for g in range(4):
                        nc.sync.dma_start(
                            wg_rows[b][:, s_off + g * Q:
                                       s_off + (g + 1) * Q],
                            w_rows[b][32 * g:32 * g + 1, qoff:qoff + Q])
                    wr = scpool.tile([128, s_len], bf16, tag="wrb")
                    nc.gpsimd.partition_broadcast(
                        wr[:], wg_rows[b][:, s_off:s_off + s_len])

                    def amr_block(ci=ci, vts=vts, wr=wr,
                                  s_len=s_len):
                        if ci == 0:
                            for k in range(NK):
                                junk = scpool.tile([128, s_len], bf16,
                                                   tag="junk")
                                nc.vector.affine_mul_reduce(
                                    junk[:], ctx_all[:, b * NK + k:
                                                     b * NK + k + 1],
                                    vts[k][:], wr[:], 1.0, 0.0)
                        else:
                            parts = scpool.tile([128, NK],
                                                mybir.dt.float32,
                                                tag="parts")
                            for k in range(NK):
                                junk = scpool.tile([128, s_len], bf16,
                                                   tag="junk")
                                nc.vector.affine_mul_reduce(
                                    junk[:], parts[:, k:k + 1],
                                    vts[k][:], wr[:], 1.0, 0.0)
                            acc = ctx_all[:, b * NK:(b + 1) * NK]
                            nc.vector.tensor_tensor(acc, acc, parts[:],
                                                    Alu.add)

                    if ci < len(chunks_for(b)) - 1:
                        amr_block()
                    else:
                        deferred_amr = amr_block

                # ---- per-batch epilogue, overlaps next batch ------------
                nc.vector.tensor_reduce(
                    lsum4[:, b:b + 1],
                    lparts[:, b * 8:(b + 1) * 8],
                    mybir.AxisListType.X, Alu.add)
                for g in range(4):
                    nc.sync.dma_start(
                        lrow4[:, 4 * b + g:4 * b + g + 1],
                        lsum4[32 * g:32 * g + 1, b:b + 1])
                nc.vector.tensor_reduce(
                    l_row[:, b:b + 1],
                    lrow4[:, 4 * b:4 * (b + 1)],
                    mybir.AxisListType.X, Alu.add)
                nc.vector.reciprocal(linv_row[:, b:b + 1],
                                     l_row[:, b:b + 1])

                lrb = wp.tile([128, 1], mybir.dt.float32, tag="wrep")
                nc.tensor.matmul(lrb[:], ones_f32[:],
                                 linv_row[:, b:b + 1],
                                 start=True, stop=True)
                nc.vector.tensor_copy(linv_rep[:, b:b + 1], lrb[:])
                nc.scalar.activation(
                    wout_rows[b][:], wg_rows[b][:], Act.Copy,
                    scale=linv_row[:, b:b + 1])
                nc.sync.dma_start(attw_out[b:b + 1, :], wout_rows[b][:])
                deferred_amr()
                ctxt = wp.tile([NK, 128], mybir.dt.float32, tag="wrep")
                nc.tensor.transpose(
                    ctxt[:], ctx_all[:, b * NK:(b + 1) * NK], ident[:])
                ctxs = scpool.tile([NK, 128], mybir.dt.float32,
                                   tag=f"ctxs{b}")
                nc.vector.tensor_scalar(
                    ctxs[:], ctxt[:], linv_rep[0:NK, b:b + 1], None,
                    Alu.mult)
                nc.sync.dma_start(
                    ctx_out[b:b + 1, :].rearrange("b (k p) -> k (b p)",
                                                  k=NK),
                    ctxs[:],
                )

    nc.compile()
    return nc


def _prep_inputs(query, values, W1, b1, W2, b2, V, bv):
    """Host-side shard prep. bv is dropped: softmax is shift-invariant."""
    w1_8 = np.ascontiguousarray(W1.astype(BF16).reshape(NK, 128, U))
    w2_8 = np.ascontiguousarray(W2.astype(BF16).reshape(NK, 128, U))
    b12 = np.ascontiguousarray((b1 + b2).astype(np.float32)
                               .reshape(NJ, 128).T)
    vv = np.ascontiguousarray(V[:, 0].astype(BF16).reshape(NJ, 128).T)
    in_maps = []
    for c in range(NCORES):
        bs = slice(c * BPC, (c + 1) * BPC)
        vTa = np.ascontiguousarray(
            np.swapaxes(values[bs].astype(BF16), 1, 2))
        qT = np.ascontiguousarray(
            query[bs].T.astype(BF16).reshape(NK, 128, BPC))
        in_maps.append({
            "vT": vTa, "w1d": w1_8, "w2d": w2_8, "qTd": qT,
            "b12d": b12, "vvd": vv,
        })
    return in_maps


def kernel(query, values, W1, b1, W2, b2, V, bv):
    global LAST_RESULTS
    from concourse.bass_utils import run_bass_kernel_spmd

    if "nc" not in _CACHE:
        _CACHE["nc"] = _build_nc()
    nc = _CACHE["nc"]

    query = np.asarray(query)
    values = np.asarray(values)
    in_maps = _prep_inputs(query, values, np.asarray(W1), np.asarray(b1),
                           np.asarray(W2), np.asarray(b2), np.asarray(V),
                           np.asarray(bv))

    res = run_bass_kernel_spmd(nc, in_maps, core_ids=list(range(NCORES)),
                               **RUN_KWARGS)
    LAST_RESULTS = res

    ctx = np.concatenate(
        [np.asarray(r["ctx_out"]) for r in res.results], axis=0)
    attw = np.concatenate(
        [np.asarray(r["attw_out"]) for r in res.results], axis=0)
    return (np.ascontiguousarray(ctx, dtype=np.float32),
            np.ascontiguousarray(attw, dtype=np.float32)[..., None])


# revision 47
# speedup vs baseline: 1.0134x; 1.0005x over previous
"""Bahdanau attention on 8 TRN2 NeuronCores, data-parallel over batch.

Reference computation (B=32, S=2048, H=U=1024):
    proj_v = values @ W1 + b1                 # [B,S,U]
    proj_q = query @ W2 + b2                  # [B,1,U]
    score  = tanh(proj_v + proj_q) @ V + bv   # [B,S,1]
    attw   = softmax(score, axis=1)
    ctx    = sum(attw * values, axis=1)       # [B,H]

Kernel strategy (per core, 4 batches):
  - values pre-transposed on host to [b, H, S] bf16 so the contraction dim
    (h) lands on SBUF partitions; W1 stays resident as the stationary matmul
    operand.  proj is produced transposed [u, s], which makes proj_q a
    per-partition bias fused into the tanh activation.
  - score via PE matvec with V over u-chunks; exp on ScalarE with accum_out
    giving the softmax denominator for free.  Scores are O(1) so the softmax
    runs unnormalized in a single pass (no max subtraction, no second read
    of values); bv drops out entirely by shift invariance.
  - exp(score) row is broadcast to 128 partitions with a rank-1 matmul and
    fused into sum(e^s * values) via the affine_mul_reduce DVE op on the
    same [h, s] tiles the big matmul consumed.
  - per-batch epilogue (softmax normalize + output DMA) overlaps the next
    batch's compute; context leaves via a PE transpose so the final DMA is
    contiguous.
"""

import numpy as np
import ml_dtypes

B, S, H, U = 32, 2048, 1024, 1024
NCORES = 8
BPC = B // NCORES  # batches per core
SC = 512           # s-chunk (matmul moving free dim)
NSC = S // SC
NK = H // 128      # contraction chunks
NJ = U // 128      # u chunks

BF16 = ml_dtypes.bfloat16

_CACHE = {}
LAST_RESULTS = None
RUN_KWARGS = {}


def _build_nc():
    import concourse.bacc as bacc
    import concourse.mybir as mybir
    import concourse.tile as tile
    from concourse import masks

    dt = mybir.dt
    f32 = dt.float32
    bf16 = dt.bfloat16
    Alu = mybir.AluOpType
    Act = mybir.ActivationFunctionType

    nc = bacc.Bacc("TRN2", target_bir_lowering=False, debug=False,
                   num_devices=NCORES)

    vT = nc.dram_tensor("vT", [BPC, H, S], bf16, kind="ExternalInput")
    w1d = nc.dram_tensor("w1d", [NK, 128, U], bf16, kind="ExternalInput")
    w2d = nc.dram_tensor("w2d", [NK, 128, U], bf16, kind="ExternalInput")
    qTd = nc.dram_tensor("qTd", [NK, 128, BPC], bf16, kind="ExternalInput")
    b12d = nc.dram_tensor("b12d", [128, NJ], f32, kind="ExternalInput")
    vvd = nc.dram_tensor("vvd", [128, NJ], bf16, kind="ExternalInput")
    ctx_out = nc.dram_tensor("ctx_out", [BPC, H], f32, kind="ExternalOutput")
    attw_out = nc.dram_tensor("attw_out", [BPC, S], f32, kind="ExternalOutput")

    def w_slice(w_sb, k, j):
        return w_sb[:, k * U + j * 128:k * U + (j + 1) * 128]

    with tile.TileContext(nc) as tc:
        with (
            tc.tile_pool(name="const", bufs=1) as cpool,
            tc.tile_pool(name="vt", bufs=24) as vtpool,
            tc.tile_pool(name="tanh", bufs=14) as thpool,
            tc.tile_pool(name="scratch", bufs=2) as scpool,
            tc.tile_pool(name="pp", bufs=5, space="PSUM") as pp,
            tc.tile_pool(name="sp", bufs=1, space="PSUM") as sp,
            tc.tile_pool(name="wp", bufs=2, space="PSUM") as wp,
        ):
            # ---- resident tiles -----------------------------------------
            w1_sb = cpool.tile([128, NK * U], bf16)
            w2_sb = cpool.tile([128, NK * U], bf16)
            qT_sb = cpool.tile([128, NK * BPC], bf16)
            b12_sb = cpool.tile([128, NJ], f32)
            vv_sb = cpool.tile([128, NJ], bf16)
            ones_bf = cpool.tile([128, 128], bf16)
            ones_f32 = cpool.tile([1, 128], f32)
            ident = cpool.tile([128, 128], f32)
            bias_full = cpool.tile([128, NJ * BPC], f32)
            w_rows = []
            wg_rows = []
            wout_rows = []
            for b in range(BPC):
                wrow = cpool.tile([128, NSC * 128], bf16, tag=f"wrow{b}")
                w_rows.append(wrow)
                wgrow = cpool.tile([1, S], bf16, tag=f"wgrow{b}")
                wg_rows.append(wgrow)
                worow = cpool.tile([1, S], f32, tag=f"worow{b}")
                wout_rows.append(worow)
            lparts = cpool.tile([128, BPC * 8], f32)
            lsum4 = cpool.tile([128, BPC], f32)
            lrow4 = cpool.tile([1, 4 * BPC], f32)
            l_row = cpool.tile([1, BPC], f32)
            linv_row = cpool.tile([1, BPC], f32)
            linv_rep = cpool.tile([128, BPC], f32)
            ctx_all = cpool.tile([128, BPC * NK], f32)

            # ---- input DMAs: W1 + first value tiles first ---------------
            vts0 = []
            for k in range(NK):
                nc.sync.dma_start(w1_sb[:, k * U:(k + 1) * U], w1d[k])
                t = vtpool.tile([128, SC], bf16, tag="vt")
                nc.sync.dma_start(t[:], vT[0, k * 128:(k + 1) * 128, 0:SC])
                vts0.append(t)
            nc.gpsimd.memset(ones_bf[:], 1.0)
            nc.gpsimd.memset(lparts[:], 0.0)
            nc.gpsimd.memset(ones_f32[:], 1.0)
            masks.make_identity(nc, ident[:])
            for k in range(NK):
                nc.sync.dma_start(qT_sb[:, k * BPC:(k + 1) * BPC], qTd[k])
            for k in range(NK):
                nc.sync.dma_start(w2_sb[:, k * U:(k + 1) * U], w2d[k])
            nc.sync.dma_start(b12_sb[:], b12d[:])
            nc.sync.dma_start(vv_sb[:], vvd[:])

            # ---- bias = W2^T q + (b1 + b2), [u_part, j*BPC + b] ----------
            for j in range(NJ):
                bp = wp.tile([128, BPC], mybir.dt.float32, tag="wrep")
                for k in range(NK):
                    nc.tensor.matmul(
                        bp[:], w_slice(w2_sb, k, j),
                        qT_sb[:, k * BPC:(k + 1) * BPC],
                        start=(k == 0), stop=(k == NK - 1),
                    )
                nc.vector.tensor_scalar(
                    bias_full[:, j * BPC:(j + 1) * BPC], bp[:],
                    b12_sb[:, j:j + 1], None, Alu.add,
                )

            # ---- main loop ----------------------------------------------
            def chunks_for(b):
                if b < BPC - 1:
                    return [(i * SC, SC) for i in range(NSC)]
                return ([(i * SC, SC) for i in range(NSC - 1)]
                        + [(S - SC, SC // 2), (S - SC // 2, SC // 2)])

            for b in range(BPC):
                for ci, (s_off, s_len) in enumerate(chunks_for(b)):
                    Q = s_len // 4
                    if b == 0 and ci == 0:
                        vts = vts0
                    else:
                        vts = []
                        for k in range(NK):
                            t = vtpool.tile([128, s_len], bf16, tag="vt")
                            nc.sync.dma_start(
                                t[:], vT[b, k * 128:(k + 1) * 128,
                                         s_off:s_off + s_len])
                            vts.append(t)

                    sps = sp.tile([128, Q], mybir.dt.float32, tag="score")
                    tanhs = []
                    nhalf = (s_len + 511) // 512
                    for j in range(NJ):
                        pj = pp.tile([128, s_len], mybir.dt.float32,
                                     tag="proj")
                        for h in range(nhalf):
                            c0, c1 = h * 512, min((h + 1) * 512, s_len)
                            for k in range(NK):
                                nc.tensor.matmul(
                                    pj[:, c0:c1], w_slice(w1_sb, k, j),
                                    vts[k][:, c0:c1],
                                    start=(k == 0), stop=(k == NK - 1),
                                )
                        tj = thpool.tile([128, s_len], bf16, tag="tanh")
                        nc.scalar.activation(
                            tj[:], pj[:], Act.Tanh,
                            bias=bias_full[:, j * BPC + b:j * BPC + b + 1],
                        )
                        tanhs.append(tj)

                    for j in range(NJ):
                        for g in range(4):
                            nc.tensor.matmul(
                                sps[32 * g:32 * g + 1, :],
                                vv_sb[:, j:j + 1],
                                tanhs[j][:, g * Q:(g + 1) * Q],
                                start=(j == 0), stop=(j == NJ - 1),
                                tile_position=(0, 32 * g),
                                skip_group_check=True,
                            )

                    qoff = s_off // 4
                    for g in range(4):
                        nc.scalar.activation(
                            w_rows[b][32 * g:32 * g + 1, qoff:qoff + Q],
                            sps[32 * g:32 * g + 1, :],
                            Act.Exp,
                            accum_out=lparts[32 * g:32 * g + 1,
                                             b * 8 + ci:b * 8 + ci + 1],
                        )
                    for g in range(4):
                        nc.sync.dma_start(
                            wg_rows[b][:, s_off + g * Q:
                                       s_off + (g + 1) * Q],
                            w_rows[b][32 * g:32 * g + 1, qoff:qoff + Q])
                    wr = scpool.tile([128, s_len], bf16, tag="wrb")
                    nc.gpsimd.partition_broadcast(
                        wr[:], wg_rows[b][:, s_off:s_off + s_len])

                    def amr_block(ci=ci, vts=vts, wr=wr,
                                  s_len=s_len):
                        if ci == 0:
                            for k in range(NK):
                                junk = scpool.tile([128, s_len], bf16,
                                                   tag="junk")
                                nc.vector.affine_mul_reduce(
                                    junk[:], ctx_all[:, b * NK + k:
                                                     b * NK + k + 1],
                                    vts[k][:], wr[:], 1.0, 0.0)
                        else:
                            parts = scpool.tile([128, NK],
                                                mybir.dt.float32,
                                                tag="parts")
                            for k in range(NK):
                                junk = scpool.tile([128, s_len], bf16,
                                                   tag="junk")
                                nc.vector.affine_mul_reduce(
                                    junk[:], parts[:, k:k + 1],
                                    vts[k][:], wr[:], 1.0, 0.0)
                            acc = ctx_all[:, b * NK:(b + 1) * NK]
                            nc.vector.tensor_tensor(acc, acc, parts[:],
                                                    Alu.add)

                    if ci < len(chunks_for(b)) - 1:
                        amr_block()
                    else:
                        deferred_amr = amr_block

                # ---- per-batch epilogue, overlaps next batch ------------
                nc.vector.tensor_reduce(
                    lsum4[:, b:b + 1],
                    lparts[:, b * 8:(b + 1) * 8],
                    mybir.AxisListType.X, Alu.add)
                for g in range(4):
                    nc.sync.dma_start(
                        lrow4[:, 4 * b + g:4 * b + g + 1],
                        lsum4[32 * g:32 * g + 1, b:b + 1])
                nc.vector.tensor_reduce(
                    l_row[:, b:b + 1],
                    lrow4[:, 4 * b:4 * (b + 1)],
                    mybir.AxisListType.X, Alu.add)
                nc.vector.reciprocal(linv_row[:, b:b + 1],
                                     l_row[:, b:b + 1])

                lrb = wp.tile([128, 1], mybir.dt.float32, tag="wrep")
                nc.tensor.matmul(lrb[:], ones_f32[:],
                                 linv_row[:, b:b + 1],
                                 start=True, stop=True)
                nc.vector.tensor_copy(linv_rep[:, b:b + 1], lrb[:])
                nc.scalar.activation(
                    wout_rows[b][:], wg_rows[b][:], Act.Copy,
                    scale=linv_row[:, b:b + 1])
                nc.sync.dma_start(attw_out[b:b + 1, :], wout_rows[b][:])
                deferred_amr()
                ctxt = wp.tile([NK, 128], mybir.dt.float32, tag="wrep")
                nc.tensor.transpose(
                    ctxt[:], ctx_all[:, b * NK:(b + 1) * NK], ident[:])
                ctxs = scpool.tile([NK, 128], mybir.dt.float32,
                                   tag=f"ctxs{b}")
                nc.vector.tensor_scalar(
                    ctxs[:], ctxt[:], linv_rep[0:NK, b:b + 1], None,
                    Alu.mult)
                nc.sync.dma_start(
                    ctx_out[b:b + 1, :].rearrange("b (k p) -> k (b p)",
                                                  k=NK),
                    ctxs[:],
                )

    nc.compile()
    return nc


def _prep_inputs(query, values, W1, b1, W2, b2, V, bv):
    """Host-side shard prep. bv is dropped: softmax is shift-invariant."""
    w1_8 = np.ascontiguousarray(W1.astype(BF16).reshape(NK, 128, U))
    w2_8 = np.ascontiguousarray(W2.astype(BF16).reshape(NK, 128, U))
    b12 = np.ascontiguousarray((b1 + b2).astype(np.float32)
                               .reshape(NJ, 128).T)
    vv = np.ascontiguousarray(V[:, 0].astype(BF16).reshape(NJ, 128).T)
    in_maps = []
    for c in range(NCORES):
        bs = slice(c * BPC, (c + 1) * BPC)
        vTa = np.ascontiguousarray(
            np.swapaxes(values[bs].astype(BF16), 1, 2))
        qT = np.ascontiguousarray(
            query[bs].T.astype(BF16).reshape(NK, 128, BPC))
        in_maps.append({
            "vT": vTa, "w1d": w1_8, "w2d": w2_8, "qTd": qT,
            "b12d": b12, "vvd": vv,
        })
    return in_maps


def kernel(query, values, W1, b1, W2, b2, V, bv):
    global LAST_RESULTS
    from concourse.bass_utils import run_bass_kernel_spmd

    if "nc" not in _CACHE:
        _CACHE["nc"] = _build_nc()
    nc = _CACHE["nc"]

    query = np.asarray(query)
    values = np.asarray(values)
    in_maps = _prep_inputs(query, values, np.asarray(W1), np.asarray(b1),
                           np.asarray(W2), np.asarray(b2), np.asarray(V),
                           np.asarray(bv))

    res = run_bass_kernel_spmd(nc, in_maps, core_ids=list(range(NCORES)),
                               **RUN_KWARGS)
    LAST_RESULTS = res

    ctx = np.concatenate(
        [np.asarray(r["ctx_out"]) for r in res.results], axis=0)
    attw = np.concatenate(
        [np.asarray(r["attw_out"]) for r in res.results], axis=0)
    return (np.ascontiguousarray(ctx, dtype=np.float32),
            np.ascontiguousarray(attw, dtype=np.float32)[..., None])


# revision 48
# speedup vs baseline: 1.0191x; 1.0057x over previous
"""Bahdanau attention on 8 TRN2 NeuronCores, data-parallel over batch.

Reference computation (B=32, S=2048, H=U=1024):
    proj_v = values @ W1 + b1                 # [B,S,U]
    proj_q = query @ W2 + b2                  # [B,1,U]
    score  = tanh(proj_v + proj_q) @ V + bv   # [B,S,1]
    attw   = softmax(score, axis=1)
    ctx    = sum(attw * values, axis=1)       # [B,H]

Kernel strategy (per core, 4 batches):
  - values pre-transposed on host to [b, H, S] bf16 so the contraction dim
    (h) lands on SBUF partitions; W1 stays resident as the stationary matmul
    operand.  proj is produced transposed [u, s], which makes proj_q a
    per-partition bias fused into the tanh activation.
  - score via PE matvec with V over u-chunks; exp on ScalarE with accum_out
    giving the softmax denominator for free.  Scores are O(1) so the softmax
    runs unnormalized in a single pass (no max subtraction, no second read
    of values); bv drops out entirely by shift invariance.
  - exp(score) row is broadcast to 128 partitions with a rank-1 matmul and
    fused into sum(e^s * values) via the affine_mul_reduce DVE op on the
    same [h, s] tiles the big matmul consumed.
  - per-batch epilogue (softmax normalize + output DMA) overlaps the next
    batch's compute; context leaves via a PE transpose so the final DMA is
    contiguous.
"""

import numpy as np
import ml_dtypes

B, S, H, U = 32, 2048, 1024, 1024
NCORES = 8
BPC = B // NCORES  # batches per core
SC = 512           # s-chunk (matmul moving free dim)
NSC = S // SC
NK = H // 128      # contraction chunks
NJ = U // 128      # u chunks

BF16 = ml_dtypes.bfloat16

_CACHE = {}
LAST_RESULTS = None
RUN_KWARGS = {}


def _build_nc():
    import concourse.bacc as bacc
    import concourse.mybir as mybir
    import concourse.tile as tile
    from concourse import masks

    dt = mybir.dt
    f32 = dt.float32
    bf16 = dt.bfloat16
    Alu = mybir.AluOpType
    Act = mybir.ActivationFunctionType

    nc = bacc.Bacc("TRN2", target_bir_lowering=False, debug=False,
                   num_devices=NCORES)

    vT = nc.dram_tensor("vT", [BPC, H, S], bf16, kind="ExternalInput")
    w1d = nc.dram_tensor("w1d", [NK, 128, U], bf16, kind="ExternalInput")
    w2d = nc.dram_tensor("w2d", [NK, 128, U], bf16, kind="ExternalInput")
    qTd = nc.dram_tensor("qTd", [NK, 128, BPC], bf16, kind="ExternalInput")
    b12d = nc.dram_tensor("b12d", [128, NJ], f32, kind="ExternalInput")
    vvd = nc.dram_tensor("vvd", [128, NJ], bf16, kind="ExternalInput")
    ctx_out = nc.dram_tensor("ctx_out", [BPC, H], f32, kind="ExternalOutput")
    attw_out = nc.dram_tensor("attw_out", [BPC, S], f32, kind="ExternalOutput")

    def w_slice(w_sb, k, j):
        return w_sb[:, k * U + j * 128:k * U + (j + 1) * 128]

    with tile.TileContext(nc) as tc:
        with (
            tc.tile_pool(name="const", bufs=1) as cpool,
            tc.tile_pool(name="vt", bufs=24) as vtpool,
            tc.tile_pool(name="tanh", bufs=14) as thpool,
            tc.tile_pool(name="scratch", bufs=2) as scpool,
            tc.tile_pool(name="pp", bufs=5, space="PSUM") as pp,
            tc.tile_pool(name="sp", bufs=1, space="PSUM") as sp,
            tc.tile_pool(name="wp", bufs=2, space="PSUM") as wp,
        ):
            # ---- resident tiles -----------------------------------------
            w1_sb = cpool.tile([128, NK * U], bf16)
            w2_sb = cpool.tile([128, NK * U], bf16)
            qT_sb = cpool.tile([128, NK * BPC], bf16)
            b12_sb = cpool.tile([128, NJ], f32)
            vv_sb = cpool.tile([128, NJ], bf16)
            ones_bf = cpool.tile([128, 128], bf16)
            ones_f32 = cpool.tile([1, 128], f32)
            ident = cpool.tile([128, 128], f32)
            bias_full = cpool.tile([128, NJ * BPC], f32)
            w_rows = []
            wg_rows = []
            wout_rows = []
            for b in range(BPC):
                wrow = cpool.tile([128, NSC * 128], bf16, tag=f"wrow{b}")
                w_rows.append(wrow)
                wgrow = cpool.tile([1, S], bf16, tag=f"wgrow{b}")
                wg_rows.append(wgrow)
                worow = cpool.tile([1, S], f32, tag=f"worow{b}")
                wout_rows.append(worow)
            lparts = cpool.tile([128, BPC * 8], f32)
            lsum4 = cpool.tile([128, BPC], f32)
            lrow4 = cpool.tile([1, 4 * BPC], f32)
            l_row = cpool.tile([1, BPC], f32)
            linv_row = cpool.tile([1, BPC], f32)
            linv_rep = cpool.tile([128, BPC], f32)
            ctx_all = cpool.tile([128, BPC * NK], f32)

            # ---- input DMAs: W1 + first value tiles first ---------------
            vts0 = []
            for k in range(NK):
                nc.sync.dma_start(w1_sb[:, k * U:(k + 1) * U], w1d[k])
                t = vtpool.tile([128, SC], bf16, tag="vt")
                nc.sync.dma_start(t[:], vT[0, k * 128:(k + 1) * 128, 0:SC])
                vts0.append(t)
            nc.gpsimd.memset(ones_bf[:], 1.0)
            nc.gpsimd.memset(lparts[:], 0.0)
            nc.gpsimd.memset(ones_f32[:], 1.0)
            masks.make_identity(nc, ident[:])
            for k in range(NK):
                nc.sync.dma_start(qT_sb[:, k * BPC:(k + 1) * BPC], qTd[k])
            for k in range(NK):
                nc.sync.dma_start(w2_sb[:, k * U:(k + 1) * U], w2d[k])
            nc.sync.dma_start(b12_sb[:], b12d[:])
            nc.sync.dma_start(vv_sb[:], vvd[:])

            # ---- bias = W2^T q + (b1 + b2), [u_part, j*BPC + b] ----------
            for j in range(NJ):
                bp = wp.tile([128, BPC], mybir.dt.float32, tag="wrep")
                for k in range(NK):
                    nc.tensor.matmul(
                        bp[:], w_slice(w2_sb, k, j),
                        qT_sb[:, k * BPC:(k + 1) * BPC],
                        start=(k == 0), stop=(k == NK - 1),
                    )
                nc.vector.tensor_scalar(
                    bias_full[:, j * BPC:(j + 1) * BPC], bp[:],
                    b12_sb[:, j:j + 1], None, Alu.add,
                )

            # ---- main loop ----------------------------------------------
            def chunks_for(b):
                if b < BPC - 1:
                    return [(i * SC, SC) for i in range(NSC)]
                return ([(i * SC, SC) for i in range(NSC - 1)]
                        + [(S - SC, SC // 2), (S - SC // 2, SC // 2)])

            for b in range(BPC):
                for ci, (s_off, s_len) in enumerate(chunks_for(b)):
                    Q = s_len // 4
                    if b == 0 and ci == 0:
                        vts = vts0
                    else:
                        vts = []
                        for k in range(NK):
                            t = vtpool.tile([128, s_len], bf16, tag="vt")
                            nc.sync.dma_start(
                                t[:], vT[b, k * 128:(k + 1) * 128,
                                         s_off:s_off + s_len])
                            vts.append(t)

                    last1 = (b == BPC - 1
                             and ci == len(chunks_for(b)) - 1)
                    sps = sp.tile([128, s_len if last1 else Q],
                                  mybir.dt.float32, tag="score")
                    tanhs = []
                    nhalf = (s_len + 511) // 512
                    for j in range(NJ):
                        pj = pp.tile([128, s_len], mybir.dt.float32,
                                     tag="proj")
                        for h in range(nhalf):
                            c0, c1 = h * 512, min((h + 1) * 512, s_len)
                            for k in range(NK):
                                nc.tensor.matmul(
                                    pj[:, c0:c1], w_slice(w1_sb, k, j),
                                    vts[k][:, c0:c1],
                                    start=(k == 0), stop=(k == NK - 1),
                                )
                        tj = thpool.tile([128, s_len], bf16, tag="tanh")
                        nc.scalar.activation(
                            tj[:], pj[:], Act.Tanh,
                            bias=bias_full[:, j * BPC + b:j * BPC + b + 1],
                        )
                        tanhs.append(tj)

                    if last1:
                        for j in range(NJ):
                            nc.tensor.matmul(
                                sps[0:1, :], vv_sb[:, j:j + 1],
                                tanhs[j][:],
                                start=(j == 0), stop=(j == NJ - 1),
                            )
                        nc.scalar.activation(
                            wg_rows[b][:, s_off:s_off + s_len],
                            sps[0:1, :], Act.Exp,
                            accum_out=lparts[0:1,
                                             b * 8 + ci:b * 8 + ci + 1],
                        )
                    else:
                        for j in range(NJ):
                            for g in range(4):
                                nc.tensor.matmul(
                                    sps[32 * g:32 * g + 1, :],
                                    vv_sb[:, j:j + 1],
                                    tanhs[j][:, g * Q:(g + 1) * Q],
                                    start=(j == 0), stop=(j == NJ - 1),
                                    tile_position=(0, 32 * g),
                                    skip_group_check=True,
                                )
                        qoff = s_off // 4
                        for g in range(4):
                            nc.scalar.activation(
                                w_rows[b][32 * g:32 * g + 1,
                                          qoff:qoff + Q],
                                sps[32 * g:32 * g + 1, :],
                                Act.Exp,
                                accum_out=lparts[32 * g:32 * g + 1,
                                                 b * 8 + ci:
                                                 b * 8 + ci + 1],
                            )
                        for g in range(4):
                            nc.sync.dma_start(
                                wg_rows[b][:, s_off + g * Q:
                                           s_off + (g + 1) * Q],
                                w_rows[b][32 * g:32 * g + 1,
                                          qoff:qoff + Q])
                    wr = scpool.tile([128, s_len], bf16, tag="wrb")
                    nc.gpsimd.partition_broadcast(
                        wr[:], wg_rows[b][:, s_off:s_off + s_len])

                    def amr_block(ci=ci, vts=vts, wr=wr,
                                  s_len=s_len):
                        if ci == 0:
                            for k in range(NK):
                                junk = scpool.tile([128, s_len], bf16,
                                                   tag="junk")
                                nc.vector.affine_mul_reduce(
                                    junk[:], ctx_all[:, b * NK + k:
                                                     b * NK + k + 1],
                                    vts[k][:], wr[:], 1.0, 0.0)
                        else:
                            parts = scpool.tile([128, NK],
                                                mybir.dt.float32,
                                                tag="parts")
                            for k in range(NK):
                                junk = scpool.tile([128, s_len], bf16,
                                                   tag="junk")
                                nc.vector.affine_mul_reduce(
                                    junk[:], parts[:, k:k + 1],
                                    vts[k][:], wr[:], 1.0, 0.0)
                            acc = ctx_all[:, b * NK:(b + 1) * NK]
                            nc.vector.tensor_tensor(acc, acc, parts[:],
                                                    Alu.add)

                    if ci < len(chunks_for(b)) - 1:
                        amr_block()
                    else:
                        deferred_amr = amr_block

                # ---- per-batch epilogue, overlaps next batch ------------
                nc.vector.tensor_reduce(
                    lsum4[:, b:b + 1],
                    lparts[:, b * 8:(b + 1) * 8],
                    mybir.AxisListType.X, Alu.add)
                for g in range(4):
                    nc.sync.dma_start(
                        lrow4[:, 4 * b + g:4 * b + g + 1],
                        lsum4[32 * g:32 * g + 1, b:b + 1])
                nc.vector.tensor_reduce(
                    l_row[:, b:b + 1],
                    lrow4[:, 4 * b:4 * (b + 1)],
                    mybir.AxisListType.X, Alu.add)
                nc.vector.reciprocal(linv_row[:, b:b + 1],
                                     l_row[:, b:b + 1])

                lrb = wp.tile([128, 1], mybir.dt.float32, tag="wrep")
                nc.tensor.matmul(lrb[:], ones_f32[:],
                                 linv_row[:, b:b + 1],
                                 start=True, stop=True)
                nc.vector.tensor_copy(linv_rep[:, b:b + 1], lrb[:])
                nc.scalar.activation(
                    wout_rows[b][:], wg_rows[b][:], Act.Copy,
                    scale=linv_row[:, b:b + 1])
                nc.sync.dma_start(attw_out[b:b + 1, :], wout_rows[b][:])
                deferred_amr()
                ctxt = wp.tile([NK, 128], mybir.dt.float32, tag="wrep")
                nc.tensor.transpose(
                    ctxt[:], ctx_all[:, b * NK:(b + 1) * NK], ident[:])
                ctxs = scpool.tile([NK, 128], mybir.dt.float32,
                                   tag=f"ctxs{b}")
                nc.vector.tensor_scalar(
                    ctxs[:], ctxt[:], linv_rep[0:NK, b:b + 1], None,
                    Alu.mult)
                nc.sync.dma_start(
                    ctx_out[b:b + 1, :].rearrange("b (k p) -> k (b p)",
                                                  k=NK),
                    ctxs[:],
                )

    nc.compile()
    return nc


def _prep_inputs(query, values, W1, b1, W2, b2, V, bv):
    """Host-side shard prep. bv is dropped: softmax is shift-invariant."""
    w1_8 = np.ascontiguousarray(W1.astype(BF16).reshape(NK, 128, U))
    w2_8 = np.ascontiguousarray(W2.astype(BF16).reshape(NK, 128, U))
    b12 = np.ascontiguousarray((b1 + b2).astype(np.float32)
                               .reshape(NJ, 128).T)
    vv = np.ascontiguousarray(V[:, 0].astype(BF16).reshape(NJ, 128).T)
    in_maps = []
    for c in range(NCORES):
        bs = slice(c * BPC, (c + 1) * BPC)
        vTa = np.ascontiguousarray(
            np.swapaxes(values[bs].astype(BF16), 1, 2))
        qT = np.ascontiguousarray(
            query[bs].T.astype(BF16).reshape(NK, 128, BPC))
        in_maps.append({
            "vT": vTa, "w1d": w1_8, "w2d": w2_8, "qTd": qT,
            "b12d": b12, "vvd": vv,
        })
    return in_maps


def kernel(query, values, W1, b1, W2, b2, V, bv):
    global LAST_RESULTS
    from concourse.bass_utils import run_bass_kernel_spmd

    if "nc" not in _CACHE:
        _CACHE["nc"] = _build_nc()
    nc = _CACHE["nc"]

    query = np.asarray(query)
    values = np.asarray(values)
    in_maps = _prep_inputs(query, values, np.asarray(W1), np.asarray(b1),
                           np.asarray(W2), np.asarray(b2), np.asarray(V),
                           np.asarray(bv))

    res = run_bass_kernel_spmd(nc, in_maps, core_ids=list(range(NCORES)),
                               **RUN_KWARGS)
    LAST_RESULTS = res

    ctx = np.concatenate(
        [np.asarray(r["ctx_out"]) for r in res.results], axis=0)
    attw = np.concatenate(
        [np.asarray(r["attw_out"]) for r in res.results], axis=0)
    return (np.ascontiguousarray(ctx, dtype=np.float32),
            np.ascontiguousarray(attw, dtype=np.float32)[..., None])


# revision 49
# speedup vs baseline: 1.0229x; 1.0037x over previous
"""Bahdanau attention on 8 TRN2 NeuronCores, data-parallel over batch.

Reference computation (B=32, S=2048, H=U=1024):
    proj_v = values @ W1 + b1                 # [B,S,U]
    proj_q = query @ W2 + b2                  # [B,1,U]
    score  = tanh(proj_v + proj_q) @ V + bv   # [B,S,1]
    attw   = softmax(score, axis=1)
    ctx    = sum(attw * values, axis=1)       # [B,H]

Kernel strategy (per core, 4 batches):
  - values pre-transposed on host to [b, H, S] bf16 so the contraction dim
    (h) lands on SBUF partitions; W1 stays resident as the stationary matmul
    operand.  proj is produced transposed [u, s], which makes proj_q a
    per-partition bias fused into the tanh activation.
  - score via PE matvec with V over u-chunks; exp on ScalarE with accum_out
    giving the softmax denominator for free.  Scores are O(1) so the softmax
    runs unnormalized in a single pass (no max subtraction, no second read
    of values); bv drops out entirely by shift invariance.
  - exp(score) row is broadcast to 128 partitions with a rank-1 matmul and
    fused into sum(e^s * values) via the affine_mul_reduce DVE op on the
    same [h, s] tiles the big matmul consumed.
  - per-batch epilogue (softmax normalize + output DMA) overlaps the next
    batch's compute; context leaves via a PE transpose so the final DMA is
    contiguous.
"""

import numpy as np
import ml_dtypes

B, S, H, U = 32, 2048, 1024, 1024
NCORES = 8
BPC = B // NCORES  # batches per core
SC = 512           # s-chunk (matmul moving free dim)
NSC = S // SC
NK = H // 128      # contraction chunks
NJ = U // 128      # u chunks

BF16 = ml_dtypes.bfloat16

_CACHE = {}
LAST_RESULTS = None
RUN_KWARGS = {}


def _build_nc():
    import concourse.bacc as bacc
    import concourse.mybir as mybir
    import concourse.tile as tile
    from concourse import masks

    dt = mybir.dt
    f32 = dt.float32
    bf16 = dt.bfloat16
    Alu = mybir.AluOpType
    Act = mybir.ActivationFunctionType

    nc = bacc.Bacc("TRN2", target_bir_lowering=False, debug=False,
                   num_devices=NCORES)

    vT = nc.dram_tensor("vT", [BPC, H, S], bf16, kind="ExternalInput")
    w1d = nc.dram_tensor("w1d", [NK, 128, U], bf16, kind="ExternalInput")
    w2d = nc.dram_tensor("w2d", [NK, 128, U], bf16, kind="ExternalInput")
    qTd = nc.dram_tensor("qTd", [NK, 128, BPC], bf16, kind="ExternalInput")
    b12d = nc.dram_tensor("b12d", [128, NJ], f32, kind="ExternalInput")
    vvd = nc.dram_tensor("vvd", [128, NJ], bf16, kind="ExternalInput")
    ctx_out = nc.dram_tensor("ctx_out", [BPC, H], f32, kind="ExternalOutput")
    attw_out = nc.dram_tensor("attw_out", [BPC, S], f32, kind="ExternalOutput")

    def w_slice(w_sb, k, j):
        return w_sb[:, k * U + j * 128:k * U + (j + 1) * 128]

    with tile.TileContext(nc) as tc:
        with (
            tc.tile_pool(name="const", bufs=1) as cpool,
            tc.tile_pool(name="vt", bufs=24) as vtpool,
            tc.tile_pool(name="tanh", bufs=14) as thpool,
            tc.tile_pool(name="scratch", bufs=2) as scpool,
            tc.tile_pool(name="pp", bufs=5, space="PSUM") as pp,
            tc.tile_pool(name="sp", bufs=1, space="PSUM") as sp,
            tc.tile_pool(name="wp", bufs=2, space="PSUM") as wp,
        ):
            # ---- resident tiles -----------------------------------------
            w1_sb = cpool.tile([128, NK * U], bf16)
            w2_sb = cpool.tile([128, NK * U], bf16)
            qT_sb = cpool.tile([128, NK * BPC], bf16)
            b12_sb = cpool.tile([128, NJ], f32)
            vv_sb = cpool.tile([128, NJ], bf16)
            ones_bf = cpool.tile([128, 128], bf16)
            ones_f32 = cpool.tile([1, 128], f32)
            ident = cpool.tile([128, 128], f32)
            bias_full = cpool.tile([128, NJ * BPC], f32)
            w_rows = []
            wg_rows = []
            wout_rows = []
            for b in range(BPC):
                wrow = cpool.tile([128, NSC * 128], bf16, tag=f"wrow{b}")
                w_rows.append(wrow)
                wgrow = cpool.tile([1, S], bf16, tag=f"wgrow{b}")
                wg_rows.append(wgrow)
                worow = cpool.tile([1, S], f32, tag=f"worow{b}")
                wout_rows.append(worow)
            lparts = cpool.tile([128, BPC * 8], f32)
            lsum4 = cpool.tile([128, BPC], f32)
            lrow4 = cpool.tile([1, 4 * BPC], f32)
            l_row = cpool.tile([1, BPC], f32)
            linv_row = cpool.tile([1, BPC], f32)
            linv_rep = cpool.tile([128, BPC], f32)
            ctx_all = cpool.tile([128, BPC * NK], f32)

            # ---- input DMAs: W1 + first value tiles first ---------------
            vts0 = []
            for k in range(NK):
                nc.sync.dma_start(w1_sb[:, k * U:(k + 1) * U], w1d[k])
                t = vtpool.tile([128, SC], bf16, tag="vt")
                nc.sync.dma_start(t[:], vT[0, k * 128:(k + 1) * 128, 0:SC])
                vts0.append(t)
            nc.gpsimd.memset(ones_bf[:], 1.0)
            nc.gpsimd.memset(lparts[:], 0.0)
            nc.gpsimd.memset(ones_f32[:], 1.0)
            masks.make_identity(nc, ident[:])
            for k in range(NK):
                nc.sync.dma_start(qT_sb[:, k * BPC:(k + 1) * BPC], qTd[k])
            for k in range(NK):
                nc.sync.dma_start(w2_sb[:, k * U:(k + 1) * U], w2d[k])
            nc.sync.dma_start(b12_sb[:], b12d[:])
            nc.sync.dma_start(vv_sb[:], vvd[:])

            # ---- bias = W2^T q + (b1 + b2), [u_part, j*BPC + b] ----------
            for j in range(NJ):
                bp = wp.tile([128, BPC], mybir.dt.float32, tag="wrep")
                for k in range(NK):
                    nc.tensor.matmul(
                        bp[:], w_slice(w2_sb, k, j),
                        qT_sb[:, k * BPC:(k + 1) * BPC],
                        start=(k == 0), stop=(k == NK - 1),
                    )
                nc.vector.tensor_scalar(
                    bias_full[:, j * BPC:(j + 1) * BPC], bp[:],
                    b12_sb[:, j:j + 1], None, Alu.add,
                )

            # ---- main loop ----------------------------------------------
            def chunks_for(b):
                if b < BPC - 1:
                    return [(i * SC, SC) for i in range(NSC)]
                return ([(i * SC, SC) for i in range(NSC - 1)]
                        + [(S - SC, SC // 2), (S - SC // 2, SC // 2)])

            pending_ctx = []
            for b in range(BPC):
                for ci, (s_off, s_len) in enumerate(chunks_for(b)):
                    Q = s_len // 4
                    if b == 0 and ci == 0:
                        vts = vts0
                    else:
                        vts = []
                        for k in range(NK):
                            t = vtpool.tile([128, s_len], bf16, tag="vt")
                            nc.sync.dma_start(
                                t[:], vT[b, k * 128:(k + 1) * 128,
                                         s_off:s_off + s_len])
                            vts.append(t)

                    last1 = (b == BPC - 1
                             and ci == len(chunks_for(b)) - 1)
                    sps = sp.tile([128, s_len if last1 else Q],
                                  mybir.dt.float32, tag="score")
                    tanhs = []
                    nhalf = (s_len + 511) // 512
                    for j in range(NJ):
                        pj = pp.tile([128, s_len], mybir.dt.float32,
                                     tag="proj")
                        for h in range(nhalf):
                            c0, c1 = h * 512, min((h + 1) * 512, s_len)
                            for k in range(NK):
                                nc.tensor.matmul(
                                    pj[:, c0:c1], w_slice(w1_sb, k, j),
                                    vts[k][:, c0:c1],
                                    start=(k == 0), stop=(k == NK - 1),
                                )
                        tj = thpool.tile([128, s_len], bf16, tag="tanh")
                        nc.scalar.activation(
                            tj[:], pj[:], Act.Tanh,
                            bias=bias_full[:, j * BPC + b:j * BPC + b + 1],
                        )
                        tanhs.append(tj)

                    if last1:
                        for j in range(NJ):
                            nc.tensor.matmul(
                                sps[0:1, :], vv_sb[:, j:j + 1],
                                tanhs[j][:],
                                start=(j == 0), stop=(j == NJ - 1),
                            )
                        nc.scalar.activation(
                            wg_rows[b][:, s_off:s_off + s_len],
                            sps[0:1, :], Act.Exp,
                            accum_out=lparts[0:1,
                                             b * 8 + ci:b * 8 + ci + 1],
                        )
                    else:
                        for j in range(NJ):
                            for g in range(4):
                                nc.tensor.matmul(
                                    sps[32 * g:32 * g + 1, :],
                                    vv_sb[:, j:j + 1],
                                    tanhs[j][:, g * Q:(g + 1) * Q],
                                    start=(j == 0), stop=(j == NJ - 1),
                                    tile_position=(0, 32 * g),
                                    skip_group_check=True,
                                )
                        qoff = s_off // 4
                        for g in range(4):
                            nc.scalar.activation(
                                w_rows[b][32 * g:32 * g + 1,
                                          qoff:qoff + Q],
                                sps[32 * g:32 * g + 1, :],
                                Act.Exp,
                                accum_out=lparts[32 * g:32 * g + 1,
                                                 b * 8 + ci:
                                                 b * 8 + ci + 1],
                            )
                        for g in range(4):
                            nc.sync.dma_start(
                                wg_rows[b][:, s_off + g * Q:
                                           s_off + (g + 1) * Q],
                                w_rows[b][32 * g:32 * g + 1,
                                          qoff:qoff + Q])
                    wr = scpool.tile([128, s_len], bf16, tag="wrb")
                    nc.gpsimd.partition_broadcast(
                        wr[:], wg_rows[b][:, s_off:s_off + s_len])

                    def amr_block(ci=ci, vts=vts, wr=wr,
                                  s_len=s_len):
                        if ci == 0:
                            for k in range(NK):
                                junk = scpool.tile([128, s_len], bf16,
                                                   tag="junk")
                                nc.vector.affine_mul_reduce(
                                    junk[:], ctx_all[:, b * NK + k:
                                                     b * NK + k + 1],
                                    vts[k][:], wr[:], 1.0, 0.0)
                        else:
                            parts = scpool.tile([128, NK],
                                                mybir.dt.float32,
                                                tag="parts")
                            for k in range(NK):
                                junk = scpool.tile([128, s_len], bf16,
                                                   tag="junk")
                                nc.vector.affine_mul_reduce(
                                    junk[:], parts[:, k:k + 1],
                                    vts[k][:], wr[:], 1.0, 0.0)
                            acc = ctx_all[:, b * NK:(b + 1) * NK]
                            nc.vector.tensor_tensor(acc, acc, parts[:],
                                                    Alu.add)

                    if ci < len(chunks_for(b)) - 1:
                        amr_block()
                    else:
                        deferred_amr = amr_block
                    if ci == 0 and pending_ctx:
                        pending_ctx.pop(0)()

                # ---- per-batch epilogue, overlaps next batch ------------
                nc.vector.tensor_reduce(
                    lsum4[:, b:b + 1],
                    lparts[:, b * 8:(b + 1) * 8],
                    mybir.AxisListType.X, Alu.add)
                for g in range(4):
                    nc.sync.dma_start(
                        lrow4[:, 4 * b + g:4 * b + g + 1],
                        lsum4[32 * g:32 * g + 1, b:b + 1])
                nc.vector.tensor_reduce(
                    l_row[:, b:b + 1],
                    lrow4[:, 4 * b:4 * (b + 1)],
                    mybir.AxisListType.X, Alu.add)
                nc.vector.reciprocal(linv_row[:, b:b + 1],
                                     l_row[:, b:b + 1])

                lrb = wp.tile([128, 1], mybir.dt.float32, tag="wrep")
                nc.tensor.matmul(lrb[:], ones_f32[:],
                                 linv_row[:, b:b + 1],
                                 start=True, stop=True)
                nc.vector.tensor_copy(linv_rep[:, b:b + 1], lrb[:])
                nc.scalar.activation(
                    wout_rows[b][:], wg_rows[b][:], Act.Copy,
                    scale=linv_row[:, b:b + 1])
                nc.sync.dma_start(attw_out[b:b + 1, :], wout_rows[b][:])
                deferred_amr()
                ctxt = wp.tile([NK, 128], mybir.dt.float32, tag="wrep")
                nc.tensor.transpose(
                    ctxt[:], ctx_all[:, b * NK:(b + 1) * NK], ident[:])
                ctxs = scpool.tile([NK, 128], mybir.dt.float32,
                                   tag=f"ctxs{b}")
                nc.vector.tensor_scalar(
                    ctxs[:], ctxt[:], linv_rep[0:NK, b:b + 1], None,
                    Alu.mult)
                nc.sync.dma_start(
                    ctx_out[b:b + 1, :].rearrange("b (k p) -> k (b p)",
                                                  k=NK),
                    ctxs[:],
                )

    nc.compile()
    return nc


def _prep_inputs(query, values, W1, b1, W2, b2, V, bv):
    """Host-side shard prep. bv is dropped: softmax is shift-invariant."""
    w1_8 = np.ascontiguousarray(W1.astype(BF16).reshape(NK, 128, U))
    w2_8 = np.ascontiguousarray(W2.astype(BF16).reshape(NK, 128, U))
    b12 = np.ascontiguousarray((b1 + b2).astype(np.float32)
                               .reshape(NJ, 128).T)
    vv = np.ascontiguousarray(V[:, 0].astype(BF16).reshape(NJ, 128).T)
    in_maps = []
    for c in range(NCORES):
        bs = slice(c * BPC, (c + 1) * BPC)
        vTa = np.ascontiguousarray(
            np.swapaxes(values[bs].astype(BF16), 1, 2))
        qT = np.ascontiguousarray(
            query[bs].T.astype(BF16).reshape(NK, 128, BPC))
        in_maps.append({
            "vT": vTa, "w1d": w1_8, "w2d": w2_8, "qTd": qT,
            "b12d": b12, "vvd": vv,
        })
    return in_maps


def kernel(query, values, W1, b1, W2, b2, V, bv):
    global LAST_RESULTS
    from concourse.bass_utils import run_bass_kernel_spmd

    if "nc" not in _CACHE:
        _CACHE["nc"] = _build_nc()
    nc = _CACHE["nc"]

    query = np.asarray(query)
    values = np.asarray(values)
    in_maps = _prep_inputs(query, values, np.asarray(W1), np.asarray(b1),
                           np.asarray(W2), np.asarray(b2), np.asarray(V),
                           np.asarray(bv))

    res = run_bass_kernel_spmd(nc, in_maps, core_ids=list(range(NCORES)),
                               **RUN_KWARGS)
    LAST_RESULTS = res

    ctx = np.concatenate(
        [np.asarray(r["ctx_out"]) for r in res.results], axis=0)
    attw = np.concatenate(
        [np.asarray(r["attw_out"]) for r in res.results], axis=0)
    return (np.ascontiguousarray(ctx, dtype=np.float32),
            np.ascontiguousarray(attw, dtype=np.float32)[..., None])
